# revision 17
# baseline (speedup 1.0000x reference)
"""Energy contrastive ranking loss on 8 TRN2 NeuronCores.

loss = sum_{i,j: d_i < d_j, i != j} relu(e_i - e_j + 1) / max(count, 1)

Sharding: core c owns i-columns [c*1024, (c+1)*1024) of the 8192x8192 pair
matrix, with j on the SBUF partition dim (64 j-tiles of 128). Per j-tile:
  cmp  = (s_i < s_j)            {0,1} bf16 from f16-rounded squared
                                distances (4x DVE mode); accum_out gives
                                the exact pair count
  relu = max(e_i + 1 - e_j, 0)  tensor_scalar on DVE (4x) or ACT
  loss = sum cmp*relu           8 accumulating [128,128] PE matmuls per
                                tile into one PSUM tile; the diagonal of
                                sum_t cmp_t^T relu_t is the masked loss
                                per i-column (off-diagonal is discarded)
Squared distances are compared (monotone == same mask as L2 distances);
both compare operands round through f16 with identical instruction
shapes, so the diagonal compares bit-equal values and self-excludes.
f16 ties between near-equal distances drop a pair from both numerator
and denominator, which cancels to first order in the final ratio.
cmp/relu work is spread across DVE / ACT / Pool via per-tile assignment
tables; the PE covers all mask-multiply+reduce work. Per-core partial
[loss_sum, count] is summed on host and divided.
"""

import numpy as np
from contextlib import ExitStack

import concourse.bass as bass
import concourse.tile as tile
from concourse import bacc, mybir
from concourse.bass_utils import run_bass_kernel_spmd

B = 8192          # batch
K = 16            # property dim
NCORES = 8
P = 128           # partitions
T = B // P        # 64 j-tiles
W = B // NCORES   # 1024 i columns per core
CH = W // P       # 8 t-slots per 128-row chunk
NCH = T // CH     # 8 chunks
NB = W // P       # 8 column blocks per tile for the PE product matmuls
MARGIN = 1.0

F32 = mybir.dt.float32
F16 = mybir.dt.float16
BF16 = mybir.dt.bfloat16
AOP = mybir.AluOpType
AFT = mybir.ActivationFunctionType
AX = mybir.AxisListType


def _spread(n, total=T):
    # n tile indices spread evenly over range(total)
    return {t for t in range(total) if (t * n) // total < ((t + 1) * n) // total}


def _spread_excl(n, excl, total=T):
    # n tile indices spread evenly over range(total) minus excl
    avail = [t for t in range(total) if t not in excl]
    m = len(avail)
    return {avail[k] for k in range(m) if (k * n) // m < ((k + 1) * n) // m}


# ---- tunables -------------------------------------------------------------
N_CMP_POOL = 0                # of 64 cmp ops, how many go to Pool (rest DVE)
                              # NB: real-HW gpsimd is ~10x slower than the
                              # cost model claims - keep Pool out of the loop
N_RELU_ACT = 30               # of 64 relu ops, how many go to ACT (rest DVE)
N_RELU_POOL = 0               # of the non-ACT relu ops, how many go to Pool
PROD_PE = True                # products+reduce on PE (diag-matmul chain);
                              # False: fused DVE scalar_tensor_tensor accum
LOOP_BUFS = 16
REPEAT = 1                    # replicate whole body (timing harness)
STAGE = 3                     # debug bisect: 0=prologue 1=+cmp 2=+relu 3=full
# ---------------------------------------------------------------------------


def _body(ctx, tc, pv_all, pv_i, e_all, e_i, pt, eye, out):
    nc = tc.nc
    cmp_pool = _spread(N_CMP_POOL)
    relu_act = _spread(N_RELU_ACT)
    relu_pool = _spread_excl(N_RELU_POOL, relu_act)

    const = ctx.enter_context(tc.tile_pool(name="const", bufs=1))
    work = ctx.enter_context(tc.tile_pool(name="work", bufs=2))
    loop = ctx.enter_context(tc.tile_pool(name="loop", bufs=LOOP_BUFS))
    psum = ctx.enter_context(tc.tile_pool(name="psum", bufs=2,
                                          space=bass.MemorySpace.PSUM))
    psacc = ctx.enter_context(tc.tile_pool(name="psacc", bufs=1,
                                           space=bass.MemorySpace.PSUM))

    ones_row = const.tile([1, P], F32)   # k=1 matmul lhsT: broadcasts a row
    nc.vector.memset(ones_row[:], 1.0)
    ones_col = const.tile([P, 1], F32)   # partition-reduce matmul lhsT
    nc.vector.memset(ones_col[:], 1.0)

    eye_sb = const.tile([P, P], BF16)    # identity: diag extraction mask
    nc.sync.dma_start(eye_sb[:], eye[:])

    join_ctr = [0]

    def bcast_row(dst, src_row, width):
        # dst[P, width] = src_row[1, width] replicated across partitions.
        # Join through one DVE copy so the matmul needs a single sync wait
        # (the PE instruction has a tiny HW wait-slot budget).
        join_ctr[0] += 1
        j = const.tile([1, width], F32, name=f"join{join_ctr[0]}")
        nc.vector.tensor_copy(j[:], src_row[:])
        for o in range(0, width, 512):
            n = min(512, width - o)
            pb = psum.tile([P, 512], F32, tag="bcast")
            nc.tensor.matmul(pb[:, :n], ones_row[:], j[:, o:o + n],
                             start=True, stop=True)
            nc.vector.tensor_copy(dst[:, o:o + n], pb[:, :n])

    # pt broadcast to all partitions, replicated CH times along free dim
    pt_row = const.tile([1, K], F32)
    nc.sync.dma_start(pt_row[:], pt[:])
    pt_bc = const.tile([P, K], F32)
    bcast_row(pt_bc, pt_row, K)
    pt_rep = const.tile([P, CH * K], F32)
    for u in range(CH):
        nc.vector.tensor_copy(pt_rep[:, u * K:(u + 1) * K], pt_bc[:])

    # this core's i-slice distances first (bcast_s gates the whole loop);
    # instruction shapes match the j-layout pass below so the diagonal
    # compares bit-equal values and self-excludes
    pvi_t = work.tile([P, CH * K], F32, tag="pv")
    nc.sync.dma_start(pvi_t[:].rearrange("p (u k) -> p u k", k=K),
                      pv_i.rearrange("(t p) k -> p t k", p=P))
    diff_i = work.tile([P, CH * K], F32, tag="diff")
    nc.vector.tensor_tensor(diff_i[:], pvi_t[:], pt_rep[:], AOP.subtract)
    sq_i = work.tile([P, CH * K], F32, tag="sq")
    nc.vector.tensor_tensor(sq_i[:], diff_i[:], diff_i[:], AOP.mult)
    s_blk = const.tile([P, CH], F32)
    nc.vector.tensor_reduce(s_blk[:], sq_i[:].rearrange("p (u k) -> p u k", k=K),
                            AX.X, AOP.add)

    # flatten s_blk [P, CH] -> row [1, W] with row[u*P + p] = s_blk[p, u]
    s_row = const.tile([1, W], F32)
    for u in range(CH):
        nc.sync.dma_start(s_row[:, u * P:(u + 1) * P], s_blk[:, u:u + 1])
    bcast_s = const.tile([P, W], F16)
    bcast_row(bcast_s, s_row, W)

    # squared distances, j-layout "(p t)": s_col[p, t] = s[p*T + t], so the
    # pv chunk loads are contiguous per partition (CH rows of K floats)
    s_col = const.tile([P, T], F32)
    pv_r = pv_all.rearrange("(p t) k -> p t k", p=P)
    for ch in range(NCH):
        pv_t = work.tile([P, CH * K], F32, tag="pv")
        nc.sync.dma_start(pv_t[:].rearrange("p (u k) -> p u k", k=K),
                          pv_r[:, ch * CH:(ch + 1) * CH, :])
        diff = work.tile([P, CH * K], F32, tag="diff")
        nc.vector.tensor_tensor(diff[:], pv_t[:], pt_rep[:], AOP.subtract)
        sq = work.tile([P, CH * K], F32, tag="sq")
        nc.vector.tensor_tensor(sq[:], diff[:], diff[:], AOP.mult)
        nc.vector.tensor_reduce(s_col[:, ch * CH:(ch + 1) * CH],
                                sq[:].rearrange("p (u k) -> p u k", k=K),
                                AX.X, AOP.add)

    # round the scalar side identically so the diagonal stays bit-equal
    # (is_lt requires an f32 scalar operand, so round-trip through f16)
    s16 = const.tile([P, T], F16)
    nc.vector.tensor_copy(s16[:], s_col[:])
    nc.vector.tensor_copy(s_col[:], s16[:])

    e_row = const.tile([1, W], F32)
    nc.sync.dma_start(e_row[:], e_i[:])
    bcast_e = const.tile([P, W], BF16)
    bcast_row(bcast_e, e_row, W)

    # e in j-layout "(p t)" (must match s_col's row mapping) and the
    # relu bias (margin - e_j); contiguous 256B per partition
    e_col = const.tile([P, T], F32)
    nc.sync.dma_start(e_col[:], e_all.rearrange("(p t) o -> p (t o)", p=P))
    bias_e = const.tile([P, T], F32)
    nc.vector.tensor_scalar(bias_e[:], e_col[:], -1.0, MARGIN, AOP.mult, AOP.add)

    cnt_acc = const.tile([P, T], F32)
    nc.vector.memset(cnt_acc[:], 0.0)
    ps_diag = psacc.tile([P, P], F32, name="ps_diag") if PROD_PE else None
    if not PROD_PE:
        loss_acc = const.tile([P, T], F32)
        nc.vector.memset(loss_acc[:], 0.0)

    n_tiles = T if STAGE >= 1 else 0
    for t in range(n_tiles):
        cmp = loop.tile([P, W], BF16, tag="cmp")
        ceng = nc.gpsimd if t in cmp_pool else nc.vector
        ceng.tensor_scalar(cmp[:], bcast_s[:], s_col[:, t:t + 1], None,
                           AOP.is_lt, AOP.add,
                           accum_out=cnt_acc[:, t:t + 1])
        if STAGE < 2:
            continue
        relu = loop.tile([P, W], BF16, tag="relu")
        if t in relu_act:
            nc.scalar.activation(relu[:], bcast_e[:], AFT.Relu,
                                 bias=bias_e[:, t:t + 1], scale=1.0)
        else:
            reng = nc.gpsimd if t in relu_pool else nc.vector
            reng.tensor_scalar(relu[:], bcast_e[:], bias_e[:, t:t + 1],
                               0.0, AOP.add, AOP.max)
        if STAGE < 3:
            continue
        if PROD_PE:
            # masked loss partial: accumulate cmp_t^T @ relu_t; only the
            # diagonal of the sum is meaningful (per-i masked loss)
            for b in range(NB):
                nc.tensor.matmul(ps_diag[:],
                                 cmp[:, b * P:(b + 1) * P],
                                 relu[:, b * P:(b + 1) * P],
                                 start=(t == 0 and b == 0),
                                 stop=(t == T - 1 and b == NB - 1))
        else:
            # fused mask-multiply + per-partition loss reduction on DVE
            prod = loop.tile([P, W], BF16, tag="prod")
            nc.vector.scalar_tensor_tensor(prod[:], cmp[:], 1.0, relu[:],
                                           AOP.mult, AOP.mult,
                                           accum_out=loss_acc[:, t:t + 1])

    # epilogue: loss = sum(diag(ps_diag)) or sum(loss_acc); count = sum(cnt_acc)
    sums = const.tile([P, 2], F32)
    if STAGE >= 3 and PROD_PE:
        diag = const.tile([P, P], F32)
        nc.vector.tensor_tensor(diag[:], ps_diag[:], eye_sb[:], AOP.mult)
        nc.vector.tensor_reduce(sums[:, 0:1], diag[:], AX.X, AOP.add)
    elif STAGE >= 3:
        nc.vector.tensor_reduce(sums[:, 0:1], loss_acc[:], AX.X, AOP.add)
    else:
        nc.vector.memset(sums[:, 0:1], 0.0)
    nc.vector.tensor_reduce(sums[:, 1:2], cnt_acc[:], AX.X, AOP.add)
    out_ps = psum.tile([1, 2], F32, tag="outp")
    nc.tensor.matmul(out_ps[:], ones_col[:], sums[:], start=True, stop=True)
    out_sb = const.tile([1, 2], F32)
    nc.vector.tensor_copy(out_sb[:], out_ps[:])
    nc.sync.dma_start(out[:], out_sb[:])


def _build_program(repeat=None):
    nc = bacc.Bacc()
    pv_all = nc.declare_dram_parameter("pv_all", [B, K], F32, isOutput=False)
    pv_i = nc.declare_dram_parameter("pv_i", [W, K], F32, isOutput=False)
    e_all = nc.declare_dram_parameter("e_all", [B, 1], F32, isOutput=False)
    e_i = nc.declare_dram_parameter("e_i", [1, W], F32, isOutput=False)
    pt = nc.declare_dram_parameter("pt", [1, K], F32, isOutput=False)
    eye = nc.declare_dram_parameter("eye", [P, P], BF16, isOutput=False)
    out = nc.declare_dram_parameter("out", [1, 2], F32, isOutput=True)
    with tile.TileContext(nc) as tc:
        for _ in range(repeat or REPEAT):
            with ExitStack() as ctx:
                _body(ctx, tc, pv_all, pv_i, e_all, e_i, pt, eye, out)
    nc.compile()
    return nc


_nc_cache = {}
_last_results = None


def _get_nc(repeat=1):
    key = (repeat, N_CMP_POOL, N_RELU_ACT, N_RELU_POOL, PROD_PE, LOOP_BUFS,
           STAGE)
    if key not in _nc_cache:
        _nc_cache[key] = _build_program(repeat)
    return _nc_cache[key]


def _eye_bf16():
    import ml_dtypes
    return np.eye(P, dtype=ml_dtypes.bfloat16)


def make_in_maps(energies, property_values, property_targets):
    e = np.ascontiguousarray(np.asarray(energies, np.float32).reshape(B, 1))
    pv = np.ascontiguousarray(np.asarray(property_values, np.float32).reshape(B, K))
    pt = np.ascontiguousarray(np.asarray(property_targets, np.float32).reshape(1, K))
    eye = _eye_bf16()
    maps = []
    for c in range(NCORES):
        sl = slice(c * W, (c + 1) * W)
        maps.append({
            "pv_all": pv,
            "pv_i": np.ascontiguousarray(pv[sl]),
            "e_all": e,
            "e_i": np.ascontiguousarray(e[sl].reshape(1, W)),
            "pt": pt,
            "eye": eye,
        })
    return maps


def finalize(parts):
    # parts: [NCORES, 2] of (loss_sum, count) fp32 partials
    loss_sum = float(np.sum(parts[:, 0], dtype=np.float64))
    count = float(np.sum(parts[:, 1], dtype=np.float64))
    loss = np.float32(loss_sum) / np.float32(max(count, 1.0))
    return np.array([loss], dtype=np.float32)


def make_runner(energies, property_values, property_targets, repeat=1):
    """Jit once, return run() -> [NCORES, 2] partials. Mirrors the
    multi-core branch of bass2jax.run_bass_via_pjrt so repeated timed
    executions don't re-trace/re-jit."""
    import jax
    from jax.experimental.shard_map import shard_map
    from jax.sharding import Mesh, PartitionSpec
    from concourse import bass2jax, mybir as mb

    nc = _get_nc(repeat)
    in_maps = make_in_maps(energies, property_values, property_targets)
    bass2jax.install_neuronx_cc_hook()
    partition_name = (nc.partition_id_tensor.name
                      if nc.partition_id_tensor else None)
    in_names, out_names, out_avals, zero_outs = [], [], [], []
    for alloc in nc.m.functions[0].allocations:
        if not isinstance(alloc, mb.MemoryLocationSet):
            continue
        name = alloc.memorylocations[0].name
        if alloc.kind == "ExternalInput":
            if name != partition_name:
                in_names.append(name)
        elif alloc.kind == "ExternalOutput":
            shape = tuple(alloc.tensor_shape)
            dtype = mb.dt.np(alloc.dtype)
            out_names.append(name)
            out_avals.append(jax.core.ShapedArray(shape, dtype))
            zero_outs.append(np.zeros(shape, dtype))
    n_params = len(in_names)
    n_outs = len(out_avals)
    all_names = list(in_names) + list(out_names)
    if partition_name is not None:
        all_names.append(partition_name)

    def _body_fn(*args):
        operands = list(args)
        if partition_name is not None:
            operands.append(bass2jax.partition_id_tensor())
        return tuple(bass2jax._bass_exec_p.bind(
            *operands,
            out_avals=tuple(out_avals),
            in_names=tuple(all_names),
            out_names=tuple(out_names),
            lowering_input_output_aliases=(),
            sim_require_finite=True,
            sim_require_nnan=True,
            nc=nc,
        ))

    devices = jax.devices()[:NCORES]
    mesh = Mesh(np.asarray(devices), ("core",))
    in_specs = (PartitionSpec("core"),) * (n_params + n_outs)
    out_specs = (PartitionSpec("core"),) * n_outs
    # No donation: the kernel writes every element of every output, so the
    # zero-init buffers need not be aliased; this lets us device_put all
    # operands once and reuse them across timed calls.
    sharded = jax.jit(
        shard_map(_body_fn, mesh=mesh, in_specs=in_specs,
                  out_specs=out_specs, check_rep=False),
        keep_unused=True)
    from jax.sharding import NamedSharding
    sh = NamedSharding(mesh, PartitionSpec("core"))
    concat_in = [
        jax.device_put(
            np.concatenate([np.asarray(in_maps[c][nm]) for c in range(NCORES)],
                           axis=0), sh)
        for nm in in_names
    ]
    dev_zeros = [
        jax.device_put(np.zeros((NCORES * z.shape[0], *z.shape[1:]), z.dtype),
                       sh)
        for z in zero_outs
    ]

    out_idx = out_names.index("out")

    def run_async():
        return sharded(*concat_in, *dev_zeros)

    def run():
        out_arrs = run_async()
        arr = np.asarray(out_arrs[out_idx]).reshape(NCORES, 1, 2)
        return arr[:, 0, :]

    run.run_async = run_async
    run.out_idx = out_idx
    return run


def kernel(energies, property_values, property_targets, repeat=1):
    global _last_results
    nc = _get_nc(repeat)
    in_maps = make_in_maps(energies, property_values, property_targets)
    res = run_bass_kernel_spmd(nc, in_maps, list(range(NCORES)))
    _last_results = res
    parts = np.stack([r["out"][0] for r in res.results])
    return finalize(parts)


# revision 21
# speedup vs baseline: 1.0105x; 1.0105x over previous
"""Energy contrastive ranking loss on 8 TRN2 NeuronCores.

loss = sum_{i,j: d_i < d_j, i != j} relu(e_i - e_j + 1) / max(count, 1)

Sharding: core c owns i-columns [c*1024, (c+1)*1024) of the 8192x8192 pair
matrix, with j on the SBUF partition dim (64 j-tiles of 128). Per j-tile:
  cmp  = (s_i < s_j)            {0,1} bf16 from f16-rounded squared
                                distances (4x DVE mode); accum_out gives
                                the exact pair count
  relu = max(e_i + 1 - e_j, 0)  tensor_scalar on DVE (4x) or ACT
  loss = sum cmp*relu           8 accumulating [128,128] PE matmuls per
                                tile into one PSUM tile; the diagonal of
                                sum_t cmp_t^T relu_t is the masked loss
                                per i-column (off-diagonal is discarded)
Squared distances are compared (monotone == same mask as L2 distances);
both compare operands round through f16 with identical instruction
shapes, so the diagonal compares bit-equal values and self-excludes.
f16 ties between near-equal distances drop a pair from both numerator
and denominator, which cancels to first order in the final ratio.
cmp/relu work is spread across DVE / ACT / Pool via per-tile assignment
tables; the PE covers all mask-multiply+reduce work. Per-core partial
[loss_sum, count] is summed on host and divided.
"""

import numpy as np
from contextlib import ExitStack

import concourse.bass as bass
import concourse.tile as tile
from concourse import bacc, mybir
from concourse.bass_utils import run_bass_kernel_spmd

B = 8192          # batch
K = 16            # property dim
NCORES = 8
P = 128           # partitions
T = B // P        # 64 j-tiles
W = B // NCORES   # 1024 i columns per core
CH = W // P       # 8 t-slots per 128-row chunk
NCH = T // CH     # 8 chunks
NB = W // P       # 8 column blocks per tile for the PE product matmuls
MARGIN = 1.0

F32 = mybir.dt.float32
F16 = mybir.dt.float16
BF16 = mybir.dt.bfloat16
AOP = mybir.AluOpType
AFT = mybir.ActivationFunctionType
AX = mybir.AxisListType


def _spread(n, total=T):
    # n tile indices spread evenly over range(total)
    return {t for t in range(total) if (t * n) // total < ((t + 1) * n) // total}


def _spread_excl(n, excl, total=T):
    # n tile indices spread evenly over range(total) minus excl
    avail = [t for t in range(total) if t not in excl]
    m = len(avail)
    return {avail[k] for k in range(m) if (k * n) // m < ((k + 1) * n) // m}


# ---- tunables -------------------------------------------------------------
N_CMP_POOL = 0                # of 64 cmp ops, how many go to Pool (rest DVE)
                              # NB: real-HW gpsimd is ~10x slower than the
                              # cost model claims - keep Pool out of the loop
N_RELU_ACT = 30               # of 64 relu ops, how many go to ACT (rest DVE)
N_RELU_POOL = 0               # of the non-ACT relu ops, how many go to Pool
PROD_PE = True                # products+reduce on PE (diag-matmul chain);
                              # False: fused DVE scalar_tensor_tensor accum
LOOP_BUFS = 16
REPEAT = 1                    # replicate whole body (timing harness)
STAGE = 3                     # debug bisect: 0=prologue 1=+cmp 2=+relu 3=full
# ---------------------------------------------------------------------------


def _body(ctx, tc, pv_all, pv_i, e_all, e_i, pt, eye, out):
    nc = tc.nc
    cmp_pool = _spread(N_CMP_POOL)
    relu_act = _spread(N_RELU_ACT)
    relu_pool = _spread_excl(N_RELU_POOL, relu_act)

    const = ctx.enter_context(tc.tile_pool(name="const", bufs=1))
    work = ctx.enter_context(tc.tile_pool(name="work", bufs=2))
    loop = ctx.enter_context(tc.tile_pool(name="loop", bufs=LOOP_BUFS))
    psum = ctx.enter_context(tc.tile_pool(name="psum", bufs=2,
                                          space=bass.MemorySpace.PSUM))
    psacc = ctx.enter_context(tc.tile_pool(name="psacc", bufs=1,
                                           space=bass.MemorySpace.PSUM))

    ones_row = const.tile([1, P], F32)   # k=1 matmul lhsT: broadcasts a row
    nc.vector.memset(ones_row[:], 1.0)
    ones_col = const.tile([P, 1], F32)   # partition-reduce matmul lhsT
    nc.vector.memset(ones_col[:], 1.0)

    eye_sb = const.tile([P, P], BF16)    # identity: diag extraction mask
    nc.sync.dma_start(eye_sb[:], eye[:])

    join_ctr = [0]

    def bcast_row(dst, src_row, width):
        # dst[P, width] = src_row[1, width] replicated across partitions.
        # Join through one DVE copy so the matmul needs a single sync wait
        # (the PE instruction has a tiny HW wait-slot budget). Wide PSUM->SBUF
        # copies go to the ACT engine to keep them off the loaded DVE.
        join_ctr[0] += 1
        j = const.tile([1, width], F32, name=f"join{join_ctr[0]}")
        nc.vector.tensor_copy(j[:], src_row[:])
        for o in range(0, width, 512):
            n = min(512, width - o)
            pb = psum.tile([P, 512], F32, tag="bcast")
            nc.tensor.matmul(pb[:, :n], ones_row[:], j[:, o:o + n],
                             start=True, stop=True)
            if n >= 512:
                nc.scalar.copy(dst[:, o:o + n], pb[:, :n])
            else:
                nc.vector.tensor_copy(dst[:, o:o + n], pb[:, :n])

    # pt broadcast to all partitions, replicated CH times along free dim
    pt_row = const.tile([1, K], F32)
    nc.sync.dma_start(pt_row[:], pt[:])
    pt_bc = const.tile([P, K], F32)
    bcast_row(pt_bc, pt_row, K)
    pt_rep = const.tile([P, CH * K], F32)
    for u in range(CH):
        nc.vector.tensor_copy(pt_rep[:, u * K:(u + 1) * K], pt_bc[:])

    # this core's i-slice distances first (bcast_s gates the whole loop);
    # instruction shapes match the j-layout pass below so the diagonal
    # compares bit-equal values and self-excludes
    pvi_t = work.tile([P, CH * K], F32, tag="pv")
    nc.sync.dma_start(pvi_t[:].rearrange("p (u k) -> p u k", k=K),
                      pv_i.rearrange("(t p) k -> p t k", p=P))
    diff_i = work.tile([P, CH * K], F32, tag="diff")
    nc.vector.tensor_tensor(diff_i[:], pvi_t[:], pt_rep[:], AOP.subtract)
    sq_i = work.tile([P, CH * K], F32, tag="sq")
    nc.vector.tensor_tensor(sq_i[:], diff_i[:], diff_i[:], AOP.mult)
    s_blk = const.tile([P, CH], F32)
    nc.vector.tensor_reduce(s_blk[:], sq_i[:].rearrange("p (u k) -> p u k", k=K),
                            AX.X, AOP.add)

    # flatten s_blk [P, CH] -> row [1, W] with row[u*P + p] = s_blk[p, u]:
    # PE-transpose to [CH, P], then CH single-descriptor row DMAs (each
    # source row is one contiguous 512B run on one partition)
    eye32 = const.tile([P, P], F32)
    nc.vector.tensor_copy(eye32[:], eye_sb[:])
    s_tp = psum.tile([CH, P], F32, tag="stp")
    nc.tensor.transpose(s_tp[:], s_blk[:], eye32[:])
    s_tp_sb = const.tile([CH, P], F32)
    nc.scalar.copy(s_tp_sb[:], s_tp[:])
    s_row = const.tile([1, W], F32)
    for u in range(CH):
        nc.sync.dma_start(s_row[:, u * P:(u + 1) * P], s_tp_sb[u:u + 1, :])
    bcast_s = const.tile([P, W], F16)
    bcast_row(bcast_s, s_row, W)

    # squared distances, j-layout "(p t)": s_col[p, t] = s[p*T + t], so the
    # pv chunk loads are contiguous per partition (CH rows of K floats)
    s_col = const.tile([P, T], F32)
    pv_r = pv_all.rearrange("(p t) k -> p t k", p=P)
    for ch in range(NCH):
        pv_t = work.tile([P, CH * K], F32, tag="pv")
        nc.sync.dma_start(pv_t[:].rearrange("p (u k) -> p u k", k=K),
                          pv_r[:, ch * CH:(ch + 1) * CH, :])
        diff = work.tile([P, CH * K], F32, tag="diff")
        nc.vector.tensor_tensor(diff[:], pv_t[:], pt_rep[:], AOP.subtract)
        sq = work.tile([P, CH * K], F32, tag="sq")
        nc.vector.tensor_tensor(sq[:], diff[:], diff[:], AOP.mult)
        nc.vector.tensor_reduce(s_col[:, ch * CH:(ch + 1) * CH],
                                sq[:].rearrange("p (u k) -> p u k", k=K),
                                AX.X, AOP.add)

    # round the scalar side identically so the diagonal stays bit-equal
    # (is_lt requires an f32 scalar operand, so round-trip through f16)
    s16 = const.tile([P, T], F16)
    nc.vector.tensor_copy(s16[:], s_col[:])
    nc.vector.tensor_copy(s_col[:], s16[:])

    e_row = const.tile([1, W], F32)
    nc.sync.dma_start(e_row[:], e_i[:])
    bcast_e = const.tile([P, W], BF16)
    bcast_row(bcast_e, e_row, W)

    # e in j-layout "(p t)" (must match s_col's row mapping) and the
    # relu bias (margin - e_j); contiguous 256B per partition
    e_col = const.tile([P, T], F32)
    nc.sync.dma_start(e_col[:], e_all.rearrange("(p t) o -> p (t o)", p=P))
    bias_e = const.tile([P, T], F32)
    nc.vector.tensor_scalar(bias_e[:], e_col[:], -1.0, MARGIN, AOP.mult, AOP.add)

    cnt_acc = const.tile([P, T], F32)
    nc.vector.memset(cnt_acc[:], 0.0)
    ps_diag = psacc.tile([P, P], F32, name="ps_diag") if PROD_PE else None
    if not PROD_PE:
        loss_acc = const.tile([P, T], F32)
        nc.vector.memset(loss_acc[:], 0.0)

    n_tiles = T if STAGE >= 1 else 0
    for t in range(n_tiles):
        cmp = loop.tile([P, W], BF16, tag="cmp")
        ceng = nc.gpsimd if t in cmp_pool else nc.vector
        ceng.tensor_scalar(cmp[:], bcast_s[:], s_col[:, t:t + 1], None,
                           AOP.is_lt, AOP.add,
                           accum_out=cnt_acc[:, t:t + 1])
        if STAGE < 2:
            continue
        relu = loop.tile([P, W], BF16, tag="relu")
        if t in relu_act:
            nc.scalar.activation(relu[:], bcast_e[:], AFT.Relu,
                                 bias=bias_e[:, t:t + 1], scale=1.0)
        else:
            reng = nc.gpsimd if t in relu_pool else nc.vector
            reng.tensor_scalar(relu[:], bcast_e[:], bias_e[:, t:t + 1],
                               0.0, AOP.add, AOP.max)
        if STAGE < 3:
            continue
        if PROD_PE:
            # masked loss partial: accumulate cmp_t^T @ relu_t; only the
            # diagonal of the sum is meaningful (per-i masked loss)
            for b in range(NB):
                nc.tensor.matmul(ps_diag[:],
                                 cmp[:, b * P:(b + 1) * P],
                                 relu[:, b * P:(b + 1) * P],
                                 start=(t == 0 and b == 0),
                                 stop=(t == T - 1 and b == NB - 1))
        else:
            # fused mask-multiply + per-partition loss reduction on DVE
            prod = loop.tile([P, W], BF16, tag="prod")
            nc.vector.scalar_tensor_tensor(prod[:], cmp[:], 1.0, relu[:],
                                           AOP.mult, AOP.mult,
                                           accum_out=loss_acc[:, t:t + 1])

    # epilogue: loss = sum(diag(ps_diag)) or sum(loss_acc); count = sum(cnt_acc)
    sums = const.tile([P, 2], F32)
    if STAGE >= 3 and PROD_PE:
        diag = const.tile([P, P], F32)
        nc.vector.tensor_tensor(diag[:], ps_diag[:], eye_sb[:], AOP.mult)
        nc.vector.tensor_reduce(sums[:, 0:1], diag[:], AX.X, AOP.add)
    elif STAGE >= 3:
        nc.vector.tensor_reduce(sums[:, 0:1], loss_acc[:], AX.X, AOP.add)
    else:
        nc.vector.memset(sums[:, 0:1], 0.0)
    nc.vector.tensor_reduce(sums[:, 1:2], cnt_acc[:], AX.X, AOP.add)
    out_ps = psum.tile([1, 2], F32, tag="outp")
    nc.tensor.matmul(out_ps[:], ones_col[:], sums[:], start=True, stop=True)
    out_sb = const.tile([1, 2], F32)
    nc.vector.tensor_copy(out_sb[:], out_ps[:])
    nc.sync.dma_start(out[:], out_sb[:])


def _build_program(repeat=None):
    nc = bacc.Bacc()
    pv_all = nc.declare_dram_parameter("pv_all", [B, K], F32, isOutput=False)
    pv_i = nc.declare_dram_parameter("pv_i", [W, K], F32, isOutput=False)
    e_all = nc.declare_dram_parameter("e_all", [B, 1], F32, isOutput=False)
    e_i = nc.declare_dram_parameter("e_i", [1, W], F32, isOutput=False)
    pt = nc.declare_dram_parameter("pt", [1, K], F32, isOutput=False)
    eye = nc.declare_dram_parameter("eye", [P, P], BF16, isOutput=False)
    out = nc.declare_dram_parameter("out", [1, 2], F32, isOutput=True)
    with tile.TileContext(nc) as tc:
        for _ in range(repeat or REPEAT):
            with ExitStack() as ctx:
                _body(ctx, tc, pv_all, pv_i, e_all, e_i, pt, eye, out)
    nc.compile()
    return nc


_nc_cache = {}
_last_results = None


def _get_nc(repeat=1):
    key = (repeat, N_CMP_POOL, N_RELU_ACT, N_RELU_POOL, PROD_PE, LOOP_BUFS,
           STAGE)
    if key not in _nc_cache:
        _nc_cache[key] = _build_program(repeat)
    return _nc_cache[key]


def _eye_bf16():
    import ml_dtypes
    return np.eye(P, dtype=ml_dtypes.bfloat16)


def make_in_maps(energies, property_values, property_targets):
    e = np.ascontiguousarray(np.asarray(energies, np.float32).reshape(B, 1))
    pv = np.ascontiguousarray(np.asarray(property_values, np.float32).reshape(B, K))
    pt = np.ascontiguousarray(np.asarray(property_targets, np.float32).reshape(1, K))
    eye = _eye_bf16()
    maps = []
    for c in range(NCORES):
        sl = slice(c * W, (c + 1) * W)
        maps.append({
            "pv_all": pv,
            "pv_i": np.ascontiguousarray(pv[sl]),
            "e_all": e,
            "e_i": np.ascontiguousarray(e[sl].reshape(1, W)),
            "pt": pt,
            "eye": eye,
        })
    return maps


def finalize(parts):
    # parts: [NCORES, 2] of (loss_sum, count) fp32 partials
    loss_sum = float(np.sum(parts[:, 0], dtype=np.float64))
    count = float(np.sum(parts[:, 1], dtype=np.float64))
    loss = np.float32(loss_sum) / np.float32(max(count, 1.0))
    return np.array([loss], dtype=np.float32)


def make_runner(energies, property_values, property_targets, repeat=1):
    """Jit once, return run() -> [NCORES, 2] partials. Mirrors the
    multi-core branch of bass2jax.run_bass_via_pjrt so repeated timed
    executions don't re-trace/re-jit."""
    import jax
    from jax.experimental.shard_map import shard_map
    from jax.sharding import Mesh, PartitionSpec
    from concourse import bass2jax, mybir as mb

    nc = _get_nc(repeat)
    in_maps = make_in_maps(energies, property_values, property_targets)
    bass2jax.install_neuronx_cc_hook()
    partition_name = (nc.partition_id_tensor.name
                      if nc.partition_id_tensor else None)
    in_names, out_names, out_avals, zero_outs = [], [], [], []
    for alloc in nc.m.functions[0].allocations:
        if not isinstance(alloc, mb.MemoryLocationSet):
            continue
        name = alloc.memorylocations[0].name
        if alloc.kind == "ExternalInput":
            if name != partition_name:
                in_names.append(name)
        elif alloc.kind == "ExternalOutput":
            shape = tuple(alloc.tensor_shape)
            dtype = mb.dt.np(alloc.dtype)
            out_names.append(name)
            out_avals.append(jax.core.ShapedArray(shape, dtype))
            zero_outs.append(np.zeros(shape, dtype))
    n_params = len(in_names)
    n_outs = len(out_avals)
    all_names = list(in_names) + list(out_names)
    if partition_name is not None:
        all_names.append(partition_name)

    def _body_fn(*args):
        operands = list(args)
        if partition_name is not None:
            operands.append(bass2jax.partition_id_tensor())
        return tuple(bass2jax._bass_exec_p.bind(
            *operands,
            out_avals=tuple(out_avals),
            in_names=tuple(all_names),
            out_names=tuple(out_names),
            lowering_input_output_aliases=(),
            sim_require_finite=True,
            sim_require_nnan=True,
            nc=nc,
        ))

    devices = jax.devices()[:NCORES]
    mesh = Mesh(np.asarray(devices), ("core",))
    in_specs = (PartitionSpec("core"),) * (n_params + n_outs)
    out_specs = (PartitionSpec("core"),) * n_outs
    # No donation: the kernel writes every element of every output, so the
    # zero-init buffers need not be aliased; this lets us device_put all
    # operands once and reuse them across timed calls.
    sharded = jax.jit(
        shard_map(_body_fn, mesh=mesh, in_specs=in_specs,
                  out_specs=out_specs, check_rep=False),
        keep_unused=True)
    from jax.sharding import NamedSharding
    sh = NamedSharding(mesh, PartitionSpec("core"))
    concat_in = [
        jax.device_put(
            np.concatenate([np.asarray(in_maps[c][nm]) for c in range(NCORES)],
                           axis=0), sh)
        for nm in in_names
    ]
    dev_zeros = [
        jax.device_put(np.zeros((NCORES * z.shape[0], *z.shape[1:]), z.dtype),
                       sh)
        for z in zero_outs
    ]

    out_idx = out_names.index("out")

    def run_async():
        return sharded(*concat_in, *dev_zeros)

    def run():
        out_arrs = run_async()
        arr = np.asarray(out_arrs[out_idx]).reshape(NCORES, 1, 2)
        return arr[:, 0, :]

    run.run_async = run_async
    run.out_idx = out_idx
    return run


def kernel(energies, property_values, property_targets, repeat=1):
    global _last_results
    nc = _get_nc(repeat)
    in_maps = make_in_maps(energies, property_values, property_targets)
    res = run_bass_kernel_spmd(nc, in_maps, list(range(NCORES)))
    _last_results = res
    parts = np.stack([r["out"][0] for r in res.results])
    return finalize(parts)
